# revision 17
# baseline (speedup 1.0000x reference)
"""Trainium2 Bass kernel for FIRResample2d (upfirdn2d, up=2, down=1, pad=(2,1),
4x4 FIR kernel).

Full input x: (16, 128, 128, 128) f32 NCHW -> output (16, 128, 256, 256) f32.

Strategy
--------
Data-parallel over 8 NeuronCores: core i processes batches [2i, 2i+1]
(no cross-device communication; the op is per-(batch, channel) spatial
filtering).

Math: with up=2, pad=(2,1) and a 4-tap kernel the op is polyphase:
    out[2m]   = k[3]*x[m-1] + k[1]*x[m]
    out[2m+1] = k[2]*x[m]   + k[0]*x[m+1]
per axis (with zero boundary).  The 4x4 kernel produced by
make_kernel([1,3,3,1], gain) is rank-1 (outer(ky, kx)), so the 2-D op
separates into a horizontal 2-tap pass followed by a vertical 2-tap pass.
We SVD the given fir_kernel at runtime into rank-1 components (always
exactly 1 for this problem) and run the separable device kernel per
component.

Per-core device program (layout: partition dim = 128 channels):
  for each batch b (2), for each 16-row strip (8):
    - DMA a [128ch, 18, 128] strip (1-row halo each side) into SBUF
    - ACT engine: pre-scaled copies of the strip / intermediate
    - DVE: scalar_tensor_tensor ops compute the horizontal pass into an
      interleaved [128, 18, 256] tile, then the vertical pass into a
      [128, 32, 256] output tile
    - DMA the output strip to DRAM (32 KiB contiguous per channel)
All elementwise work runs on the DVE (fp32 tensor-tensor class ops), the
scalar/ACT engine does the pointwise scalings, DMA overlaps via
double-buffered tile pools.
"""

import numpy as np

B_FULL, C, H, W = 16, 128, 128, 128
OH, OW = 2 * H, 2 * W
N_CORES = 8
B_PER_CORE = B_FULL // N_CORES
HS = 16  # strip height (input rows per strip)

_PROG_CACHE: dict = {}
_RESULT_KEYS = None


def _split_multi_waits(nc):
    """The walrus build here supports a single sync-wait per instruction;
    hoist extra waits onto preceding same-engine NOPs (the canonical raw-bass
    idiom: standalone waits ahead of the gated instruction)."""
    import concourse.mybir as mybir

    for f in nc.m.functions:
        for bb in f.blocks:
            new_insts = []
            for inst in bb.instructions:
                si = inst.sync_info
                waits = list(si.on_wait) if si is not None else []
                if len(waits) > 1:
                    for i, w in enumerate(waits[:-1]):
                        nop = mybir.InstNoOp(
                            name=f"{inst.name}-sw{i}",
                            engine=inst.engine,
                            sync_info=mybir.SyncInfo(on_wait=[w], on_update=[]),
                        )
                        nc.register_instruction(nop, overwrite=True)
                        new_insts.append(nop)
                    si.on_wait = [waits[-1]]
                new_insts.append(inst)
            bb.instructions = new_insts


def _build_fir_program(ky, kx, b_per_core, c, h, w, hs, reps=1, loop_n=1):
    """Build the per-core Bass program for one separable component.

    ky, kx: length-4 1-D tap vectors (floats), kernel2d = outer(ky, kx).
    Input "x" [b_per_core, c, h, w] f32, output "out" [b_per_core, c, 2h, 2w].
    """
    import concourse.bass as bass
    import concourse.mybir as mybir
    from concourse.tile import TileContext

    f32 = mybir.dt.float32
    mult = mybir.AluOpType.mult
    add = mybir.AluOpType.add

    kx0, kx1, kx2, kx3 = (float(v) for v in kx)
    ky0, ky1, ky2, ky3 = (float(v) for v in ky)
    # symmetric-separable fast path: fold ky1 into the horizontal pass so the
    # vertical pass needs no pre-scaled copy of t (saves a tile + an ACT op).
    sym = kx1 == kx2 and ky1 == ky2 and ky1 != 0.0

    oh, ow = 2 * h, 2 * w
    assert h % hs == 0
    n_strips = h // hs
    hh = hs + 2  # strip rows incl. 1-row halo on each side

    nc = bass.Bass()
    x = nc.dram_tensor("x", [b_per_core, c, h, w], f32, kind="ExternalInput")
    out = nc.dram_tensor("out", [b_per_core, c, oh, ow], f32, kind="ExternalOutput")

    import contextlib

    with TileContext(nc) as tc:
        with tc.tile_pool(name="pool", bufs=2) as pool, (
            tc.For_i(0, loop_n, 1) if loop_n > 1 else contextlib.nullcontext()
        ):
            for _rep in range(reps):
                for b in range(b_per_core):
                    t_prev = None
                    for si in range(n_strips):
                        m0 = si * hs
                        first = si == 0
                        # t-row coverage this strip computes:
                        #   first strip: rows m0-1 .. m0+hs  (hh = hs+2 slots)
                        #   later strips: rows m0+1 .. m0+hs (hs slots; rows
                        #   m0-1, m0 are reused from the previous strip's t)
                        nrows = hh if first else hs
                        r_lo = m0 - 1 if first else m0 + 1
                        xtile = pool.tile(
                            [c, nrows, w],
                            f32,
                            name="xtile",
                            bufs=4 if loop_n == 1 else 3,
                        )
                        s_lo = 0
                        if r_lo < 0:
                            r_lo, s_lo = 0, 1
                        r_hi, s_hi = m0 + hs + 1, nrows
                        if r_hi > h:
                            r_hi, s_hi = h, nrows - 1
                        nc.sync.dma_start(
                            out=xtile[:, s_lo:s_hi, :], in_=x[b, :, r_lo:r_hi, :]
                        )
                        if s_lo == 1:
                            nc.gpsimd.memset(xtile[:, 0:1, :], 0.0)
                        if s_hi == nrows - 1:
                            nc.gpsimd.memset(xtile[:, nrows - 1 : nrows, :], 0.0)

                        # --- horizontal pass: t[., 2n] = kx3*x[n-1] + kx1*x[n]
                        #                      t[., 2n+1] = kx2*x[n] + kx0*x[n+1]
                        # (sym: t' = ky1 * t throughout, undone in the v-pass)
                        hs1 = ky1 if sym else 1.0
                        xs1 = pool.tile(
                            [c, nrows, w], f32, name="xs1", bufs=2 if loop_n == 1 else 1
                        )
                        nc.scalar.mul(xs1[:], xtile[:], hs1 * kx1)
                        if sym:
                            xs2 = xs1
                        else:
                            xs2 = pool.tile([c, nrows, w], f32, name="xs2")
                            nc.scalar.mul(xs2[:], xtile[:], kx2)
                        t = pool.tile([c, nrows, ow], f32, name="t")
                        nc.vector.scalar_tensor_tensor(
                            out=t[:, :, 2:ow:2],
                            in0=xtile[:, :, 0 : w - 1],
                            scalar=hs1 * kx3,
                            in1=xs1[:, :, 1:w],
                            op0=mult,
                            op1=add,
                        )
                        nc.vector.scalar_tensor_tensor(
                            out=t[:, :, 1 : ow - 2 : 2],
                            in0=xtile[:, :, 1:w],
                            scalar=hs1 * kx0,
                            in1=xs2[:, :, 0 : w - 1],
                            op0=mult,
                            op1=add,
                        )
                        # boundary columns: x[-1] = x[w] = 0
                        nc.scalar.copy(t[:, :, 0:1], xs1[:, :, 0:1])
                        nc.scalar.copy(t[:, :, ow - 1 : ow], xs2[:, :, w - 1 : w])

                        # --- vertical pass: out[2m] = ky3*t[m-1] + ky1*t[m]
                        #                    out[2m+1] = ky2*t[m] + ky0*t[m+1]
                        if sym:
                            ta = tb = t
                            vs3, vs0 = ky3 / ky1, ky0 / ky1
                        else:
                            vs3, vs0 = ky3, ky0
                            ta = pool.tile([c, nrows, ow], f32, name="ta")
                            nc.scalar.mul(ta[:], t[:], ky1)
                            tb = pool.tile([c, nrows, ow], f32, name="tb")
                            nc.scalar.mul(tb[:], t[:], ky2)
                        obuf = pool.tile(
                            [c, 2 * hs, ow],
                            f32,
                            name="obuf",
                            bufs=3 if loop_n == 1 else 2,
                        )
                        stt = nc.vector.scalar_tensor_tensor
                        tp = tpa = tpb = None
                        if first:
                            # slots 0..hh-1 = rows m0-1..m0+hs
                            stt(
                                out=obuf[:, 0 : 2 * hs : 2, :],
                                in0=t[:, 0:hs, :],
                                scalar=vs3,
                                in1=ta[:, 1 : hs + 1, :],
                                op0=mult,
                                op1=add,
                            )
                            stt(
                                out=obuf[:, 1 : 2 * hs : 2, :],
                                in0=t[:, 2 : hs + 2, :],
                                scalar=vs0,
                                in1=tb[:, 1 : hs + 1, :],
                                op0=mult,
                                op1=add,
                            )
                        else:
                            # prev strip's t/ta/tb slots holding rows m0-1, m0
                            tp, tpa, tpb = t_prev
                            pa = tp.shape[1] - 2
                            pb = tp.shape[1] - 1
                            # even rows: out[2(m0+j)] = vs3*t[m0+j-1] + ta[m0+j]
                            stt(
                                out=obuf[:, 0:1, :],
                                in0=tp[:, pa : pa + 1, :],
                                scalar=vs3,
                                in1=tpa[:, pb : pb + 1, :],
                                op0=mult,
                                op1=add,
                            )
                            stt(
                                out=obuf[:, 2:3, :],
                                in0=tp[:, pb : pb + 1, :],
                                scalar=vs3,
                                in1=ta[:, 0:1, :],
                                op0=mult,
                                op1=add,
                            )
                            stt(
                                out=obuf[:, 4 : 2 * hs : 2, :],
                                in0=t[:, 0 : hs - 2, :],
                                scalar=vs3,
                                in1=ta[:, 1 : hs - 1, :],
                                op0=mult,
                                op1=add,
                            )
                            # odd rows: out[2(m0+j)+1] = vs0*t[m0+j+1] + tb[m0+j]
                            stt(
                                out=obuf[:, 1:2, :],
                                in0=t[:, 0:1, :],
                                scalar=vs0,
                                in1=tpb[:, pb : pb + 1, :],
                                op0=mult,
                                op1=add,
                            )
                            stt(
                                out=obuf[:, 3 : 2 * hs : 2, :],
                                in0=t[:, 1:hs, :],
                                scalar=vs0,
                                in1=tb[:, 0 : hs - 1, :],
                                op0=mult,
                                op1=add,
                            )
                        t_prev = (t, ta, tb)
                        # output DMAs on the second HWDGE ring (ACT), split by
                        # row parity so the even-row store overlaps the odd-row
                        # compute and the drain tail halves
                        nc.scalar.dma_start(
                            out=out[b, :, 2 * m0 : 2 * m0 + 2 * hs : 2, :],
                            in_=obuf[:, 0 : 2 * hs : 2, :],
                        )
                        nc.scalar.dma_start(
                            out=out[b, :, 2 * m0 + 1 : 2 * m0 + 2 * hs : 2, :],
                            in_=obuf[:, 1 : 2 * hs : 2, :],
                        )
    _split_multi_waits(nc)
    return nc


def _separable_components(k2: np.ndarray):
    """Decompose a 4x4 kernel into rank-1 (ky, kx) components via SVD.

    For this problem's kernel (outer product of [1,3,3,1] taps) there is
    exactly one component; the general path is correctness insurance.
    """
    k64 = np.asarray(k2, dtype=np.float64)
    u, s, vt = np.linalg.svd(k64)
    comps = []
    if s[0] == 0.0:
        return comps
    for i in range(len(s)):
        if s[i] <= 1e-12 * s[0]:
            break
        ky = u[:, i] * np.sqrt(s[i])
        kx = vt[i] * np.sqrt(s[i])
        # sign convention: make the largest-|.| entry of ky positive
        if ky[np.argmax(np.abs(ky))] < 0:
            ky, kx = -ky, -kx
        # snap numerically-symmetric taps so the builder's fast path fires
        for v in (ky, kx):
            if abs(v[1] - v[2]) <= 1e-6 * (abs(v[1]) + abs(v[2])):
                v[1] = v[2] = (v[1] + v[2]) / 2
            if abs(v[0] - v[3]) <= 1e-6 * (abs(v[0]) + abs(v[3]) + 1e-300):
                v[0] = v[3] = (v[0] + v[3]) / 2
        comps.append((ky, kx))
    return comps


def _get_program(ky, kx, reps=1):
    key = (tuple(np.float32(v) for v in ky), tuple(np.float32(v) for v in kx), reps)
    prog = _PROG_CACHE.get(key)
    if prog is None:
        prog = _build_fir_program(ky, kx, B_PER_CORE, C, H, W, HS, reps=reps)
        _PROG_CACHE[key] = prog
    return prog


def _run_spmd(nc, x: np.ndarray) -> np.ndarray:
    from concourse.bass_utils import run_bass_kernel_spmd

    in_maps = [
        {"x": np.ascontiguousarray(x[i * B_PER_CORE : (i + 1) * B_PER_CORE])}
        for i in range(N_CORES)
    ]
    res = run_bass_kernel_spmd(nc, in_maps, core_ids=list(range(N_CORES)))
    return np.concatenate([r["out"] for r in res.results], axis=0)


def kernel(x: np.ndarray, fir_kernel: np.ndarray) -> np.ndarray:
    x = np.asarray(x, dtype=np.float32)
    k2 = np.asarray(fir_kernel, dtype=np.float32)
    assert x.shape == (B_FULL, C, H, W), x.shape
    assert k2.shape == (4, 4), k2.shape

    comps = _separable_components(k2)
    if not comps:
        return np.zeros((B_FULL, C, OH, OW), dtype=np.float32)

    acc = None
    for ky, kx in comps:
        y = _run_spmd(_get_program(ky, kx), x)
        acc = y if acc is None else acc + y
    return acc.astype(np.float32, copy=False)
